# revision 22
# baseline (speedup 1.0000x reference)
"""Causal multi-head attention (B=4, L=2048, D=1024, H=16) on 8 TRN2 NeuronCores.

Sharding: core = (batch b, head-group hg), b in 0..3, hg in 0..1; each core
computes its batch x 8 heads and the partial out-projection; host sums the two
head-group partials per batch.

Fully-pipelined single-pass structure with PE chain interleaving, built on
two HW-probed facts: (1) a pair of K=64 matmuls issued to opposite halves
of the PE array (base partitions 0/64, distinct PSUM banks, no
accumulation) runs concurrently at ~53.6 ns per N=512 MM; (2) two
independent full-K accumulation chains interleaved across two PSUM banks
run at ~95 ns per N=512 MM vs ~166 serial, because one chain's MM streams
while the other bank's accumulate-drain turns around. (Mixing array
halves within one bank's accumulation chain is a fatal HW error, and
split-K with a two-PSUM DVE merge is illegal — so pairing whole chains is
the only safe form.) Hence:
  - scores: the two heads of a pair are natively K=64; their chunk MMs
    interleave h0/h1 so each pair overlaps on opposite array halves.
  - projections (q/k/v), attn@v, out-proj: two independent full-K
    accumulation chains (q with k, v with v, head0 with head1, nh0 with
    nh1) interleave their chunk MMs across two PSUM banks.
  - softmax denominator: ones-column in v_aug; per-block lagged
    normalization; the two heads' reciprocal rows sit in adjacent
    partitions (64, 65) so one K=2 matmul against a block-diagonal
    selector broadcasts both heads' 1/denom in a single rank-2 MM.
  - weight DMA loads are batched into a few wide descriptors so the sync
    queue doesn't serialize the prologue.

kernel(**inputs) takes the full unsharded f32 inputs and returns the full
f32 output (device math bf16, host sums the two head-group partials).
"""

import numpy as np
from ml_dtypes import bfloat16

import concourse.bass as bass
import concourse.mybir as mybir
import concourse.tile as tile
from concourse import bacc
from concourse.bass_utils import run_bass_kernel_spmd

F32 = mybir.dt.float32
BF16 = mybir.dt.bfloat16

L = 2048          # sequence length
D = 1024          # model dim
HG = 8            # heads per core
DH = 64           # head dim
DHG = HG * DH     # 512, head-group width
DC = D // 128     # 8 contraction chunks for projections
LT = L // 128     # 16 key-position chunks
QB = L // 512     # 4 query blocks of 512
N_CORES = 8


# column offset of kp-chunk j's storage inside the packed causal attnT buffer
def _off(j):
    return 2048 * j - 128 * (j * (j - 1) // 2)


ATT_W = _off(LT)  # 17408 packed causal columns per head


SKIP = set()
OLD_DMA = False
OLD_PROLOGUE = True


def build_kernel(reps: int = 0, phases: str = "pao"):
    """Build the SPMD Bass program. reps>0 wraps the body in a hardware loop
    (body executed reps+1 times total) for timing."""
    nc = bacc.Bacc()

    xT = nc.dram_tensor("xT", [D, L], BF16, kind="ExternalInput")
    wqT = nc.dram_tensor("wqT", [D, DHG], BF16, kind="ExternalInput")
    wkT = nc.dram_tensor("wkT", [D, DHG], BF16, kind="ExternalInput")
    wvT = nc.dram_tensor("wvT", [D, DHG], BF16, kind="ExternalInput")
    woT = nc.dram_tensor("woT", [DHG, D], BF16, kind="ExternalInput")
    out = nc.dram_tensor("out", [L, D], BF16, kind="ExternalOutput")

    xT_r = xT[:, :].rearrange("(c p) l -> p c l", p=128)
    wqT_r = wqT[:, :].rearrange("(c p) m -> p c m", p=128)
    wkT_r = wkT[:, :].rearrange("(c p) m -> p c m", p=128)
    wvT_r = wvT[:, :].rearrange("(c p) m -> p c m", p=128)
    woT_r = woT[:, :].rearrange("(c p) n -> p c n", p=128)
    out_r = out[:, :].rearrange("(t p) n -> p t n", p=128)

    with tile.TileContext(nc) as tc:
        ctx_body(nc, tc, xT_r, wqT_r, wkT_r, wvT_r, woT_r, out_r, reps, phases)
    nc.compile()
    return nc


def ctx_body(nc, tc, xT_r, wqT_r, wkT_r, wvT_r, woT_r, out_r, reps, phases="pao"):
    from contextlib import ExitStack

    with ExitStack() as es:
        persist = es.enter_context(tc.tile_pool(name="persist", bufs=1))
        mask_sb = persist.tile([128, 128], BF16)  # upper-tri (incl diag) ones
        ones_sb = persist.tile([128, 128], BF16)  # rank-1 denom broadcast
        nc.vector.memset(ones_sb, 1.0)

        # constant setup (outside the timing loop)
        # mask[kp, qp] = 1 where kp <= qp else 0
        nc.gpsimd.memset(mask_sb, 1.0)
        nc.gpsimd.affine_select(
            out=mask_sb,
            in_=mask_sb,
            compare_op=mybir.AluOpType.is_ge,
            fill=0.0,
            base=0,
            pattern=[[1, 128]],
            channel_multiplier=-1,
        )

        def body():
            with ExitStack() as bs:
                glob = bs.enter_context(tc.tile_pool(name="glob", bufs=1))
                qkp = bs.enter_context(tc.tile_pool(name="qkp", bufs=2))
                att = bs.enter_context(tc.tile_pool(name="att", bufs=2))
                nrm = bs.enter_context(tc.tile_pool(name="nrm", bufs=1))
                oev = bs.enter_context(tc.tile_pool(name="oev", bufs=1))
                wkps = bs.enter_context(
                    tc.tile_pool(name="wkps", bufs=1, space="PSUM")
                )

                xT_sb = glob.tile([128, DC, L], BF16)
                wq_sb = glob.tile([128, DC, DHG], BF16)
                wk_sb = glob.tile([128, DC, DHG], BF16)
                wv_sb = glob.tile([128, DC, DHG], BF16)
                wo_sb = glob.tile([128, 4, D], BF16)
                v_sb = glob.tile([128, LT, HG, DH + 1], BF16)
                outT_sb = glob.tile([128, 4, L], BF16)

                if OLD_DMA:
                    for c in range(DC):
                        nc.sync.dma_start(out=xT_sb[:, c, :], in_=xT_r[:, c, :])
                        nc.sync.dma_start(
                            out=wq_sb[:, c, 0:256], in_=wqT_r[:, c, 0:256]
                        )
                        nc.sync.dma_start(
                            out=wk_sb[:, c, 0:256], in_=wkT_r[:, c, 0:256]
                        )
                    for c in range(DC):
                        nc.sync.dma_start(
                            out=wq_sb[:, c, 256:DHG], in_=wqT_r[:, c, 256:DHG]
                        )
                        nc.sync.dma_start(
                            out=wk_sb[:, c, 256:DHG], in_=wkT_r[:, c, 256:DHG]
                        )
                    for c in range(DC):
                        nc.sync.dma_start(out=wv_sb[:, c, :], in_=wvT_r[:, c, :])
                    nc.sync.dma_start(out=wo_sb, in_=woT_r)
                else:
                    # batched loads; issue order tracks first use: x chunk 0
                    # + head-pair-0 columns of wq/wk gate the prologue
                    nc.sync.dma_start(out=xT_sb[:, 0, :], in_=xT_r[:, 0, :])
                    nc.sync.dma_start(
                        out=wq_sb[:, :, 0:128], in_=wqT_r[:, :, 0:128]
                    )
                    nc.sync.dma_start(
                        out=wk_sb[:, :, 0:128], in_=wkT_r[:, :, 0:128]
                    )
                    for c in range(1, DC):
                        nc.sync.dma_start(out=xT_sb[:, c, :], in_=xT_r[:, c, :])
                    nc.sync.dma_start(
                        out=wq_sb[:, :, 128:DHG], in_=wqT_r[:, :, 128:DHG]
                    )
                    nc.sync.dma_start(
                        out=wk_sb[:, :, 128:DHG], in_=wkT_r[:, :, 128:DHG]
                    )
                    nc.sync.dma_start(out=wv_sb[:, :, :], in_=wvT_r[:, :, :])
                    nc.sync.dma_start(out=wo_sb, in_=woT_r)

                # ones column of v_aug (denominator row of attn@v output)
                nc.vector.memset(v_sb[:, :, :, DH : DH + 1], 1.0)

                # A projection chain spec: (stationary(c, half),
                # moving(c, half), evac(ps)). Two independent chains are
                # cross-paired (X-top, Y-bottom, X-bottom, Y-top per chunk)
                # so adjacent MMs occupy opposite array halves while each
                # PSUM bank accumulates its chain's full K contraction —
                # no two-PSUM merge needed, evac stays a plain copy.
                def _hs(half):
                    if half is None:
                        return slice(0, 128)
                    return slice(half * 64, half * 64 + 64)

                def q_spec(w_sb, t, qb, dst):
                    ts = slice(t * 128, (t + 1) * 128)
                    qs = slice(qb * 512, (qb + 1) * 512)

                    def stat(c, half):
                        return w_sb[_hs(half), c, ts]

                    def mov(c, half):
                        return xT_sb[_hs(half), c, qs]

                    def evac(ps):
                        nc.vector.tensor_copy(dst[:, qs], ps)

                    return (stat, mov, evac)

                def v_spec(it):
                    its = slice(it * 128, (it + 1) * 128)

                    def stat(c, half):
                        return xT_sb[_hs(half), c, its]

                    def mov(c, half):
                        return wv_sb[_hs(half), c, :]

                    def evac(ps):
                        nc.vector.tensor_copy(
                            v_sb[:, it, :, 0:DH],
                            ps.rearrange("p (h d) -> p h d", h=HG),
                        )

                    return (stat, mov, evac)

                def chain_pair(specX, specY):
                    # two independent full-K chains interleaved across two
                    # PSUM banks: while one bank's accumulate-drain turns
                    # around, the other chain's MM streams (HW-measured
                    # 95 ns vs 166 ns per N=512 MM for a lone chain)
                    (statX, movX, evacX) = specX
                    (statY, movY, evacY) = specY
                    psX = wkps.tile([128, 512], F32, tag="w5", bufs=4)
                    psY = wkps.tile([128, 512], F32, tag="w5", bufs=4)
                    for c in range(DC):
                        st, sp = (c == 0), (c == DC - 1)
                        nc.tensor.matmul(
                            psX, statX(c, None), movX(c, None), start=st, stop=sp
                        )
                        nc.tensor.matmul(
                            psY, statY(c, None), movY(c, None), start=st, stop=sp
                        )
                    evacX(psX)
                    evacY(psY)

                def chain_full(spec):
                    # lone chain: unsplit full-K accumulation (no partner
                    # to pair array halves with)
                    (stat, mov, evac) = spec
                    ps = wkps.tile([128, 512], F32, tag="w5", bufs=4)
                    for c in range(DC):
                        nc.tensor.matmul(
                            ps,
                            stat(c, None),
                            mov(c, None),
                            start=(c == 0),
                            stop=(c == DC - 1),
                        )
                    evac(ps)

                def op_group(qt0):
                    # out-projection partials for query tiles qt0..qt0+3;
                    # nh0/nh1 chains cross-paired (nh0-top with nh1-bottom)
                    # so adjacent MMs occupy opposite array halves
                    for qt in range(qt0, qt0 + 4):
                        ot = oev.tile([128, D], BF16, tag="ot", bufs=2)
                        psA = wkps.tile([128, 512], F32, tag="w5", bufs=4)
                        psB = wkps.tile([128, 512], F32, tag="w5", bufs=4)
                        for c in range(4):
                            qs = slice(qt * 128, (qt + 1) * 128)
                            nc.tensor.matmul(
                                psA,
                                outT_sb[:, c, qs],
                                wo_sb[:, c, 0:512],
                                start=(c == 0),
                                stop=(c == 3),
                            )
                            nc.tensor.matmul(
                                psB,
                                outT_sb[:, c, qs],
                                wo_sb[:, c, 512:1024],
                                start=(c == 0),
                                stop=(c == 3),
                            )
                        nc.vector.tensor_copy(ot[:, 0:512], psA)
                        nc.vector.tensor_copy(ot[:, 512:1024], psB)
                        nc.sync.dma_start(out=out_r[:, qt, :], in_=ot)

                def sc_group(hp, b, atl, cq, ck, fillers):
                    # scores + exp for j-group 4b..4b+3; the two heads'
                    # K=64 MMs interleave so each pair runs on opposite
                    # array halves; fillers interleaved per j
                    fillers = list(fillers)
                    for j in range(4 * b, 4 * b + 4):
                        ncols = L - 128 * j
                        for c0 in range(0, ncols, 1024):
                            w = min(1024, ncols - c0)
                            if "sc" in SKIP:
                                continue
                            ps0 = wkps.tile([128, 1024], F32, tag="wk", bufs=2)
                            ps1 = wkps.tile([128, 1024], F32, tag="wk", bufs=2)
                            pst = (ps0, ps1)
                            for s0 in range(0, w, 512):
                                sw = min(512, w - s0)
                                q0 = 128 * j + c0 + s0
                                for hh in range(2):
                                    p0 = hh * 64
                                    nc.tensor.matmul(
                                        pst[hh][:, s0 : s0 + sw],
                                        ck[p0 : p0 + 64, j * 128 : (j + 1) * 128],
                                        cq[p0 : p0 + 64, q0 : q0 + sw],
                                        start=True,
                                        stop=True,
                                    )
                            if "exp" not in SKIP:
                                for hh in range(2):
                                    nc.scalar.activation(
                                        atl[hh][
                                            :, _off(j) + c0 : _off(j) + c0 + w
                                        ],
                                        pst[hh][:, :w],
                                        mybir.ActivationFunctionType.Exp,
                                        scale=0.125,
                                    )
                        if "sc" not in SKIP and "exp" not in SKIP:
                            # mask the diagonal block of this j (DVE; the
                            # one-block av lag gives it plenty of slack)
                            for hh in range(2):
                                nc.vector.tensor_mul(
                                    atl[hh][:, _off(j) : _off(j) + 128],
                                    atl[hh][:, _off(j) : _off(j) + 128],
                                    mask_sb,
                                )
                        if fillers:
                            fillers.pop(0)()
                    for f in fillers:
                        f()

                # per-(hp,b) normalization operands awaiting their lagged tail
                pending = {}

                def av_chains(hp, b, atl):
                    # attn @ v_aug chains for qp-block b, both heads: the
                    # two heads' chains cross-pair (h0-top, h1-bottom),
                    # (h0-bottom, h1-top) so adjacent MMs sit on opposite
                    # array halves while each bank accumulates its full
                    # kp contraction. Reciprocal + SBUF evac here; the
                    # rank-2 broadcast + normalize multiplies run two
                    # blocks later (norm_tail).
                    if "av" in SKIP:
                        return
                    jmax = 4 * b + 3
                    h0 = 2 * hp
                    psA = wkps.tile([128, 512], F32, tag="w5", bufs=4)
                    psB = wkps.tile([128, 512], F32, tag="w5", bufs=4)
                    pss = (psA, psB)
                    qp0 = 512 * b
                    for j in range(jmax + 1):
                        lo = max(qp0, 128 * j)
                        w = qp0 + 512 - lo
                        a0 = _off(j) + lo - 128 * j
                        st = j == 0
                        sp = j == jmax
                        for ha in range(2):
                            nc.tensor.matmul(
                                pss[ha][0 : DH + 1, lo - qp0 : 512],
                                v_sb[:, j, h0 + ha, :],
                                atl[ha][:, a0 : a0 + w],
                                start=st,
                                stop=sp,
                            )
                    recips, usts = [], []
                    for hh in range(2):
                        recip = nrm.tile([128, 512], BF16, tag="recip", bufs=5)
                        with nc.allow_low_precision(
                            reason="bf16 reciprocal feeds rank-1 denominator "
                            "broadcast; 0.4% scale noise is within tolerance"
                        ):
                            nc.vector.reciprocal(
                                recip[DH : DH + 1, :], pss[hh][DH : DH + 1, :]
                            )
                        recips.append(recip)
                    for hh in range(2):
                        ust = nrm.tile([128, 512], F32, tag="ust", bufs=5)
                        nc.vector.tensor_copy(ust[0:DH, :], pss[hh][0:DH, :])
                        usts.append(ust)
                    ust2 = nrm.tile([128, 512], F32, tag="ust2", bufs=3)
                    nc.sync.dma_start(out=ust2[DH:128, :], in_=usts[1][0:DH, :])
                    pending[(hp, b)] = (recips, usts, ust2)

                def norm_tail(hp, b):
                    # rank-1 denominator broadcast (PE) + normalize multiplies
                    # (DVE) for block b, consuming operands prepared two
                    # blocks ago
                    if (hp, b) not in pending:
                        return
                    recips, usts, ust2 = pending.pop((hp, b))
                    dst = outT_sb[:, hp, b * 512 : (b + 1) * 512]
                    rep = wkps.tile([128, 512], F32, tag="w5", bufs=4)
                    nc.tensor.matmul(
                        rep[0:DH, :],
                        ones_sb[DH : DH + 1, 0:DH],
                        recips[0][DH : DH + 1, :],
                        start=True,
                        stop=True,
                    )
                    nc.tensor.matmul(
                        rep[DH:128, :],
                        ones_sb[DH : DH + 1, DH:128],
                        recips[1][DH : DH + 1, :],
                        start=True,
                        stop=True,
                    )
                    nc.vector.tensor_mul(dst[0:DH, :], usts[0][0:DH, :], rep[0:DH, :])
                    nc.vector.tensor_mul(
                        dst[DH:128, :], ust2[DH:128, :], rep[DH:128, :]
                    )

                # prologue: head-pair 0 needs its full q (scores are key-major:
                # every j reads all query columns >= 128j) but only the first
                # k block; k blocks 1..3 ride along as attention fillers.
                # Chunk-major: blocks (q0,q1) cross-pair in ppa's two banks
                # and (q2,q3) in ppb's, so the PE streams every x chunk as
                # its DMA lands with adjacent MMs on opposite array halves;
                # the k block follows as a lone full-K chain.
                cq = qkp.tile([128, L], BF16, tag="qT", bufs=2)
                ck = qkp.tile([128, L], BF16, tag="kT", bufs=2)
                ppa = wkps.tile([128, 1024], F32, tag="wk", bufs=2, name="ppa")
                ppb = wkps.tile([128, 1024], F32, tag="wk", bufs=2, name="ppb")
                if OLD_PROLOGUE:
                    ppk = wkps.tile([128, 512], F32, tag="w5", bufs=4, name="ppk")
                    pjobs0 = [
                        (ppa[:, 0:512], wq_sb, 0),
                        (ppa[:, 512:1024], wq_sb, 1),
                        (ppb[:, 0:512], wq_sb, 2),
                        (ppb[:, 512:1024], wq_sb, 3),
                        (ppk[:, :], wk_sb, 0),
                    ]
                    for c in range(DC):
                        for ps, w_sb, qb in pjobs0:
                            nc.tensor.matmul(
                                ps,
                                w_sb[:, c, 0:128],
                                xT_sb[:, c, qb * 512 : (qb + 1) * 512],
                                start=(c == 0),
                                stop=(c == DC - 1),
                            )
                    nc.vector.tensor_copy(cq[:, 0:1024], ppa)
                    nc.vector.tensor_copy(cq[:, 1024:2048], ppb)
                    nc.vector.tensor_copy(ck[:, 0:512], ppk)
                else:
                    # (psum region, row-half, query block, first-of-region)
                    pjobs = [
                        (ppa[:, 0:512], 0, 0, True),
                        (ppa[:, 512:1024], 1, 1, True),
                        (ppa[:, 0:512], 1, 0, False),
                        (ppa[:, 512:1024], 0, 1, False),
                        (ppb[:, 0:512], 0, 2, True),
                        (ppb[:, 512:1024], 1, 3, True),
                        (ppb[:, 0:512], 1, 2, False),
                        (ppb[:, 512:1024], 0, 3, False),
                    ]
                    for c in range(DC):
                        for ps, half, qb, first in pjobs:
                            p0 = half * 64
                            nc.tensor.matmul(
                                ps,
                                wq_sb[p0 : p0 + 64, c, 0:128],
                                xT_sb[p0 : p0 + 64, c, qb * 512 : (qb + 1) * 512],
                                start=(c == 0 and first),
                                stop=(c == DC - 1 and not first),
                            )
                    nc.vector.tensor_copy(cq[:, 0:1024], ppa)
                    nc.vector.tensor_copy(cq[:, 1024:2048], ppb)
                    chain_full(q_spec(wk_sb, 0, 0, ck))

                if "a" in phases:
                    for hp in range(4):
                        at0 = att.tile([128, ATT_W], BF16, tag="attnT", bufs=2)
                        at1 = att.tile([128, ATT_W], BF16, tag="attnT", bufs=2)
                        atl = (at0, at1)
                        if hp < 3:
                            nq = qkp.tile([128, L], BF16, tag="qT", bufs=2)
                            nk = qkp.tile([128, L], BF16, tag="kT", bufs=2)
                        for b in range(QB):
                            fillers = []
                            if hp == 0:
                                # head-pair 0's own next k block feeds block
                                # b+1's scores; hp1's q/k and the v chunks
                                # (av(b-1) consumers, one block of lag)
                                # ride along, paired so every chain has an
                                # opposite-half partner
                                if b == 0:
                                    fillers = [
                                        lambda: chain_full(
                                            q_spec(wk_sb, 0, 1, ck)
                                        ),
                                        lambda: chain_pair(
                                            q_spec(wq_sb, 1, 0, nq),
                                            q_spec(wk_sb, 1, 0, nk),
                                        ),
                                    ]
                                else:
                                    v0 = 4 * (b - 1)
                                    pairs = [
                                        (
                                            q_spec(wq_sb, 1, b, nq),
                                            q_spec(wk_sb, 1, b, nk),
                                        ),
                                        (v_spec(v0), v_spec(v0 + 1)),
                                        (v_spec(v0 + 2), v_spec(v0 + 3)),
                                    ]
                                    if b < 3:
                                        pairs.append(
                                            (
                                                q_spec(wk_sb, 0, b + 1, ck),
                                                v_spec(11 + b),
                                            )
                                        )
                                    else:
                                        pairs.append(
                                            (v_spec(14), v_spec(15))
                                        )
                                    fillers = [
                                        (lambda p=p: chain_pair(*p))
                                        for p in pairs
                                    ]
                            elif hp < 3:
                                fillers = [
                                    lambda qb=b, t=hp + 1: chain_pair(
                                        q_spec(wq_sb, t, qb, nq),
                                        q_spec(wk_sb, t, qb, nk),
                                    )
                                ]
                            sc_group(hp, b, atl, cq, ck, fillers)
                            if b > 0:
                                # one-block lag: this av's exp finished while
                                # block b's scores ran
                                av_chains(hp, b - 1, atl)
                            if b > 1:
                                # two-block lag for the normalization tail
                                norm_tail(hp, b - 2)
                            if hp == 3 and "o" in phases and b == 3:
                                op_group(0)
                        av_chains(hp, 3, atl)
                        norm_tail(hp, 2)
                        if hp == 3 and "o" in phases:
                            op_group(4)
                        norm_tail(hp, 3)
                        if hp < 3:
                            cq, ck = nq, nk
                    if "o" in phases:
                        op_group(8)
                        op_group(12)
                else:
                    # keep projections live when attention is ablated
                    nc.sync.dma_start(
                        out=out_r[:, 0, 0:512], in_=cq[:, 0:512]
                    )

        if reps > 0:
            with tc.For_i(0, reps):
                body()
        body()


_CACHE = {}


def _get_runner(reps=0):
    if reps not in _CACHE:
        _CACHE[reps] = build_kernel(reps)
    return _CACHE[reps]


def make_in_maps(x, Wq, Wk, Wv, Wo):
    in_maps = []
    for core in range(N_CORES):
        b, hg = divmod(core, 2)
        sl = slice(hg * DHG, (hg + 1) * DHG)
        in_maps.append(
            {
                "xT": np.ascontiguousarray(np.asarray(x)[b].T.astype(bfloat16)),
                "wqT": np.ascontiguousarray(np.asarray(Wq)[sl, :].T.astype(bfloat16)),
                "wkT": np.ascontiguousarray(np.asarray(Wk)[sl, :].T.astype(bfloat16)),
                "wvT": np.ascontiguousarray(np.asarray(Wv)[sl, :].T.astype(bfloat16)),
                "woT": np.ascontiguousarray(np.asarray(Wo)[:, sl].T.astype(bfloat16)),
            }
        )
    return in_maps


def kernel(x, Wq, Wk, Wv, Wo):
    x = np.asarray(x)
    nc = _get_runner(0)
    in_maps = make_in_maps(x, Wq, Wk, Wv, Wo)
    res = run_bass_kernel_spmd(nc, in_maps, core_ids=list(range(N_CORES)))
    B = x.shape[0]
    out = np.empty((B, L, D), dtype=np.float32)
    for b in range(B):
        out[b] = res.results[2 * b]["out"].astype(np.float32) + res.results[
            2 * b + 1
        ]["out"].astype(np.float32)
    return out


# revision 29
# speedup vs baseline: 1.0040x; 1.0040x over previous
"""Causal multi-head attention (B=4, L=2048, D=1024, H=16) on 8 TRN2 NeuronCores.

Sharding: core = (batch b, head-group hg), b in 0..3, hg in 0..1; each core
computes its batch x 8 heads and the partial out-projection; host sums the two
head-group partials per batch.

Fully-pipelined single-pass structure with PE chain interleaving, built on
two HW-probed facts: (1) a pair of K=64 matmuls issued to opposite halves
of the PE array (base partitions 0/64, distinct PSUM banks, no
accumulation) runs concurrently at ~53.6 ns per N=512 MM; (2) two
independent full-K accumulation chains interleaved across two PSUM banks
run at ~95 ns per N=512 MM vs ~166 serial, because one chain's MM streams
while the other bank's accumulate-drain turns around. (Mixing array
halves within one bank's accumulation chain is a fatal HW error, and
split-K with a two-PSUM DVE merge is illegal — so pairing whole chains is
the only safe form.) Hence:
  - scores: the two heads of a pair are natively K=64; their chunk MMs
    interleave h0/h1 so each pair overlaps on opposite array halves.
  - projections (q/k/v), attn@v, out-proj: two independent full-K
    accumulation chains (q with k, v with v, head0 with head1, nh0 with
    nh1) interleave their chunk MMs across two PSUM banks.
  - softmax denominator: ones-column in v_aug; per-block lagged
    normalization; the two heads' reciprocal rows sit in adjacent
    partitions (64, 65) so one K=2 matmul against a block-diagonal
    selector broadcasts both heads' 1/denom in a single rank-2 MM.
  - weight DMA loads are batched into a few wide descriptors so the sync
    queue doesn't serialize the prologue.

kernel(**inputs) takes the full unsharded f32 inputs and returns the full
f32 output (device math bf16, host sums the two head-group partials).
"""

import numpy as np
from ml_dtypes import bfloat16

import concourse.bass as bass
import concourse.mybir as mybir
import concourse.tile as tile
from concourse import bacc
from concourse.bass_utils import run_bass_kernel_spmd

F32 = mybir.dt.float32
BF16 = mybir.dt.bfloat16

L = 2048          # sequence length
D = 1024          # model dim
HG = 8            # heads per core
DH = 64           # head dim
DHG = HG * DH     # 512, head-group width
DC = D // 128     # 8 contraction chunks for projections
LT = L // 128     # 16 key-position chunks
QB = L // 512     # 4 query blocks of 512
N_CORES = 8


# column offset of kp-chunk j's storage inside the packed causal attnT buffer
def _off(j):
    return 2048 * j - 128 * (j * (j - 1) // 2)


ATT_W = _off(LT)  # 17408 packed causal columns per head


SKIP = set()
OLD_DMA = False
OLD_PROLOGUE = True


def build_kernel(reps: int = 0, phases: str = "pao"):
    """Build the SPMD Bass program. reps>0 wraps the body in a hardware loop
    (body executed reps+1 times total) for timing."""
    nc = bacc.Bacc()

    xT = nc.dram_tensor("xT", [D, L], BF16, kind="ExternalInput")
    wqT = nc.dram_tensor("wqT", [D, DHG], BF16, kind="ExternalInput")
    wkT = nc.dram_tensor("wkT", [D, DHG], BF16, kind="ExternalInput")
    wvT = nc.dram_tensor("wvT", [D, DHG], BF16, kind="ExternalInput")
    woT = nc.dram_tensor("woT", [DHG, D], BF16, kind="ExternalInput")
    out = nc.dram_tensor("out", [L, D], BF16, kind="ExternalOutput")

    xT_r = xT[:, :].rearrange("(c p) l -> p c l", p=128)
    wqT_r = wqT[:, :].rearrange("(c p) m -> p c m", p=128)
    wkT_r = wkT[:, :].rearrange("(c p) m -> p c m", p=128)
    wvT_r = wvT[:, :].rearrange("(c p) m -> p c m", p=128)
    woT_r = woT[:, :].rearrange("(c p) n -> p c n", p=128)
    out_r = out[:, :].rearrange("(t p) n -> p t n", p=128)

    with tile.TileContext(nc) as tc:
        ctx_body(nc, tc, xT_r, wqT_r, wkT_r, wvT_r, woT_r, out_r, reps, phases)
    nc.compile()
    return nc


def ctx_body(nc, tc, xT_r, wqT_r, wkT_r, wvT_r, woT_r, out_r, reps, phases="pao"):
    from contextlib import ExitStack

    with ExitStack() as es:
        persist = es.enter_context(tc.tile_pool(name="persist", bufs=1))
        mask_sb = persist.tile([128, 128], BF16)  # upper-tri (incl diag) ones
        ones_sb = persist.tile([128, 128], BF16)  # rank-1 denom broadcast
        nc.vector.memset(ones_sb, 1.0)

        # constant setup (outside the timing loop)
        # mask[kp, qp] = 1 where kp <= qp else 0
        nc.gpsimd.memset(mask_sb, 1.0)
        nc.gpsimd.affine_select(
            out=mask_sb,
            in_=mask_sb,
            compare_op=mybir.AluOpType.is_ge,
            fill=0.0,
            base=0,
            pattern=[[1, 128]],
            channel_multiplier=-1,
        )

        def body():
            with ExitStack() as bs:
                glob = bs.enter_context(tc.tile_pool(name="glob", bufs=1))
                qkp = bs.enter_context(tc.tile_pool(name="qkp", bufs=2))
                att = bs.enter_context(tc.tile_pool(name="att", bufs=2))
                nrm = bs.enter_context(tc.tile_pool(name="nrm", bufs=1))
                oev = bs.enter_context(tc.tile_pool(name="oev", bufs=1))
                wkps = bs.enter_context(
                    tc.tile_pool(name="wkps", bufs=1, space="PSUM")
                )

                xT_sb = glob.tile([128, DC, L], BF16)
                wq_sb = glob.tile([128, DC, DHG], BF16)
                wk_sb = glob.tile([128, DC, DHG], BF16)
                wv_sb = glob.tile([128, DC, DHG], BF16)
                wo_sb = glob.tile([128, 4, D], BF16)
                v_sb = glob.tile([128, LT, HG, DH + 1], BF16)
                outT_sb = glob.tile([128, 4, L], BF16)

                if OLD_DMA:
                    for c in range(DC):
                        nc.sync.dma_start(out=xT_sb[:, c, :], in_=xT_r[:, c, :])
                        nc.sync.dma_start(
                            out=wq_sb[:, c, 0:256], in_=wqT_r[:, c, 0:256]
                        )
                        nc.sync.dma_start(
                            out=wk_sb[:, c, 0:256], in_=wkT_r[:, c, 0:256]
                        )
                    for c in range(DC):
                        nc.sync.dma_start(
                            out=wq_sb[:, c, 256:DHG], in_=wqT_r[:, c, 256:DHG]
                        )
                        nc.sync.dma_start(
                            out=wk_sb[:, c, 256:DHG], in_=wkT_r[:, c, 256:DHG]
                        )
                    for c in range(DC):
                        nc.sync.dma_start(out=wv_sb[:, c, :], in_=wvT_r[:, c, :])
                    nc.sync.dma_start(out=wo_sb, in_=woT_r)
                else:
                    # batched loads; issue order tracks first use: x chunk 0
                    # + head-pair-0 columns of wq/wk gate the prologue
                    nc.sync.dma_start(out=xT_sb[:, 0, :], in_=xT_r[:, 0, :])
                    nc.sync.dma_start(
                        out=wq_sb[:, :, 0:128], in_=wqT_r[:, :, 0:128]
                    )
                    nc.sync.dma_start(
                        out=wk_sb[:, :, 0:128], in_=wkT_r[:, :, 0:128]
                    )
                    for c in range(1, DC):
                        nc.sync.dma_start(out=xT_sb[:, c, :], in_=xT_r[:, c, :])
                    nc.sync.dma_start(
                        out=wq_sb[:, :, 128:DHG], in_=wqT_r[:, :, 128:DHG]
                    )
                    nc.sync.dma_start(
                        out=wk_sb[:, :, 128:DHG], in_=wkT_r[:, :, 128:DHG]
                    )
                    nc.sync.dma_start(out=wv_sb[:, :, :], in_=wvT_r[:, :, :])
                    nc.sync.dma_start(out=wo_sb, in_=woT_r)

                # ones column of v_aug (denominator row of attn@v output)
                nc.vector.memset(v_sb[:, :, :, DH : DH + 1], 1.0)

                _psctr = [0]

                def next_psname():
                    _psctr[0] += 1
                    return f"pcu{_psctr[0]}"

                # A projection chain spec: (stationary(c, half),
                # moving(c, half), evac(ps)). Two independent chains are
                # cross-paired (X-top, Y-bottom, X-bottom, Y-top per chunk)
                # so adjacent MMs occupy opposite array halves while each
                # PSUM bank accumulates its chain's full K contraction —
                # no two-PSUM merge needed, evac stays a plain copy.
                def _hs(half):
                    if half is None:
                        return slice(0, 128)
                    return slice(half * 64, half * 64 + 64)

                def q_spec(w_sb, t, qb, dst):
                    ts = slice(t * 128, (t + 1) * 128)
                    qs = slice(qb * 512, (qb + 1) * 512)

                    def stat(c, half):
                        return w_sb[_hs(half), c, ts]

                    def mov(c, half):
                        return xT_sb[_hs(half), c, qs]

                    def evac(ps):
                        nc.vector.tensor_copy(dst[:, qs], ps)

                    return (stat, mov, evac)

                def v_spec(it):
                    its = slice(it * 128, (it + 1) * 128)

                    def stat(c, half):
                        return xT_sb[_hs(half), c, its]

                    def mov(c, half):
                        return wv_sb[_hs(half), c, :]

                    def evac(ps):
                        nc.vector.tensor_copy(
                            v_sb[:, it, :, 0:DH],
                            ps.rearrange("p (h d) -> p h d", h=HG),
                        )

                    return (stat, mov, evac)

                def pair_units(specX, specY):
                    # two independent full-K chains interleaved across two
                    # PSUM banks (HW-measured 95 ns vs 166 ns per N=512 MM
                    # for a lone chain), sliced into two ~0.8us micro-units
                    (statX, movX, evacX) = specX
                    (statY, movY, evacY) = specY
                    state = {}

                    def mk(c0, c1, first, last):
                        def f():
                            if first:
                                state["X"] = wkps.tile(
                                    [128, 512], F32, tag="w5", bufs=4,
                                    name=next_psname(),
                                )
                                state["Y"] = wkps.tile(
                                    [128, 512], F32, tag="w5", bufs=4,
                                    name=next_psname(),
                                )
                            for c in range(c0, c1):
                                st, sp = (c == 0), (c == DC - 1)
                                nc.tensor.matmul(
                                    state["X"],
                                    statX(c, None),
                                    movX(c, None),
                                    start=st,
                                    stop=sp,
                                )
                                nc.tensor.matmul(
                                    state["Y"],
                                    statY(c, None),
                                    movY(c, None),
                                    start=st,
                                    stop=sp,
                                )
                            if last:
                                evacX(state["X"])
                                evacY(state["Y"])

                        return f

                    return [mk(0, 4, True, False), mk(4, DC, False, True)]

                def full_units(spec):
                    # lone chain: unsplit full-K accumulation (no partner)
                    (stat, mov, evac) = spec

                    def f():
                        ps = wkps.tile([128, 512], F32, tag="w5", bufs=4)
                        for c in range(DC):
                            nc.tensor.matmul(
                                ps,
                                stat(c, None),
                                mov(c, None),
                                start=(c == 0),
                                stop=(c == DC - 1),
                            )
                        evac(ps)

                    return [f]

                def op_unit(qt):
                    # out-projection partial for one query tile: nh0/nh1
                    # chains interleaved in two banks; evacuations split
                    # between DVE and ScalarE (ScalarE is idle once the
                    # exps are done, and these land mostly in the tail)
                    def f():
                        ot = oev.tile([128, D], BF16, tag="ot", bufs=2)
                        psA = wkps.tile([128, 512], F32, tag="w5", bufs=4)
                        psB = wkps.tile([128, 512], F32, tag="w5", bufs=4)
                        qs = slice(qt * 128, (qt + 1) * 128)
                        for c in range(4):
                            nc.tensor.matmul(
                                psA,
                                outT_sb[:, c, qs],
                                wo_sb[:, c, 0:512],
                                start=(c == 0),
                                stop=(c == 3),
                            )
                            nc.tensor.matmul(
                                psB,
                                outT_sb[:, c, qs],
                                wo_sb[:, c, 512:1024],
                                start=(c == 0),
                                stop=(c == 3),
                            )
                        nc.vector.tensor_copy(ot[:, 0:512], psA)
                        nc.scalar.activation(
                            ot[:, 512:1024],
                            psB,
                            mybir.ActivationFunctionType.Copy,
                        )
                        nc.sync.dma_start(out=out_r[:, qt, :], in_=ot)

                    return f

                def sc_group(hp, b, atl, cq, ck, units):
                    # scores + exp for j-group 4b..4b+3; the two heads'
                    # K=64 MMs interleave so each pair runs on opposite
                    # array halves. One micro-unit of other PE work pops
                    # after every score chunk so the PE keeps the exp
                    # conveyor fed without long foreign chains blocking
                    # the in-order queue; leftovers return to the caller.
                    for j in range(4 * b, 4 * b + 4):
                        ncols = L - 128 * j
                        for c0 in range(0, ncols, 1024):
                            w = min(1024, ncols - c0)
                            if "sc" in SKIP:
                                continue
                            ps0 = wkps.tile([128, 1024], F32, tag="wk", bufs=2)
                            ps1 = wkps.tile([128, 1024], F32, tag="wk", bufs=2)
                            pst = (ps0, ps1)
                            for s0 in range(0, w, 512):
                                sw = min(512, w - s0)
                                q0 = 128 * j + c0 + s0
                                for hh in range(2):
                                    p0 = hh * 64
                                    nc.tensor.matmul(
                                        pst[hh][:, s0 : s0 + sw],
                                        ck[p0 : p0 + 64, j * 128 : (j + 1) * 128],
                                        cq[p0 : p0 + 64, q0 : q0 + sw],
                                        start=True,
                                        stop=True,
                                    )
                            if "exp" not in SKIP:
                                for hh in range(2):
                                    nc.scalar.activation(
                                        atl[hh][
                                            :, _off(j) + c0 : _off(j) + c0 + w
                                        ],
                                        pst[hh][:, :w],
                                        mybir.ActivationFunctionType.Exp,
                                        scale=0.125,
                                    )
                            if units:
                                units.pop(0)()
                        if "sc" not in SKIP and "exp" not in SKIP:
                            # mask the diagonal block of this j (DVE; the
                            # one-block av lag gives it plenty of slack)
                            for hh in range(2):
                                nc.vector.tensor_mul(
                                    atl[hh][:, _off(j) : _off(j) + 128],
                                    atl[hh][:, _off(j) : _off(j) + 128],
                                    mask_sb,
                                )
                    return units

                # per-(hp,b) normalization operands awaiting their lagged tail
                pending = {}

                def av_units(hp, b, atl, seg=2):
                    # attn @ v_aug chains for qp-block b, both heads,
                    # interleaved across two PSUM banks and sliced into
                    # ~1us micro-units so they interleave between score
                    # chunks without starving the exp conveyor. The last
                    # unit computes the denominator reciprocals and
                    # evacuates the un-normalized rows; the rank-1
                    # broadcast + normalize multiplies run two blocks
                    # later (norm_tail).
                    if "av" in SKIP:
                        return []
                    jmax = 4 * b + 3
                    h0 = 2 * hp
                    qp0 = 512 * b
                    state = {}

                    def mk(js_seg, first, last):
                        def f():
                            if first:
                                state["A"] = wkps.tile(
                                    [128, 512], F32, tag="w5", bufs=4,
                                    name=f"avA_{hp}_{b}",
                                )
                                state["B"] = wkps.tile(
                                    [128, 512], F32, tag="w5", bufs=4,
                                    name=f"avB_{hp}_{b}",
                                )
                            pss = (state["A"], state["B"])
                            for j in js_seg:
                                lo = max(qp0, 128 * j)
                                w = qp0 + 512 - lo
                                a0 = _off(j) + lo - 128 * j
                                for ha in range(2):
                                    nc.tensor.matmul(
                                        pss[ha][0 : DH + 1, lo - qp0 : 512],
                                        v_sb[:, j, h0 + ha, :],
                                        atl[ha][:, a0 : a0 + w],
                                        start=(j == 0),
                                        stop=(j == jmax),
                                    )
                            if last:
                                recips, usts = [], []
                                for hh in range(2):
                                    recip = nrm.tile(
                                        [128, 512], BF16, tag="recip", bufs=5
                                    )
                                    with nc.allow_low_precision(
                                        reason="bf16 reciprocal feeds rank-1 "
                                        "denominator broadcast; 0.4% scale "
                                        "noise is within tolerance"
                                    ):
                                        nc.vector.reciprocal(
                                            recip[DH : DH + 1, :],
                                            pss[hh][DH : DH + 1, :],
                                        )
                                    recips.append(recip)
                                for hh in range(2):
                                    ust = nrm.tile(
                                        [128, 512], F32, tag="ust", bufs=5
                                    )
                                    nc.vector.tensor_copy(
                                        ust[0:DH, :], pss[hh][0:DH, :]
                                    )
                                    usts.append(ust)
                                ust2 = nrm.tile(
                                    [128, 512], F32, tag="ust2", bufs=3
                                )
                                nc.sync.dma_start(
                                    out=ust2[DH:128, :], in_=usts[1][0:DH, :]
                                )
                                pending[(hp, b)] = (recips, usts, ust2)

                        return f

                    segs = [
                        list(range(s, min(s + seg, jmax + 1)))
                        for s in range(0, jmax + 1, seg)
                    ]
                    return [
                        mk(s, i == 0, i == len(segs) - 1)
                        for i, s in enumerate(segs)
                    ]

                def norm_tail(hp, b):
                    # rank-1 denominator broadcast (PE) + normalize multiplies
                    # (DVE) for block b, consuming operands prepared two
                    # blocks ago
                    if (hp, b) not in pending:
                        return
                    recips, usts, ust2 = pending.pop((hp, b))
                    dst = outT_sb[:, hp, b * 512 : (b + 1) * 512]
                    rep = wkps.tile([128, 512], F32, tag="w5", bufs=4)
                    nc.tensor.matmul(
                        rep[0:DH, :],
                        ones_sb[DH : DH + 1, 0:DH],
                        recips[0][DH : DH + 1, :],
                        start=True,
                        stop=True,
                    )
                    nc.tensor.matmul(
                        rep[DH:128, :],
                        ones_sb[DH : DH + 1, DH:128],
                        recips[1][DH : DH + 1, :],
                        start=True,
                        stop=True,
                    )
                    nc.vector.tensor_mul(dst[0:DH, :], usts[0][0:DH, :], rep[0:DH, :])
                    nc.vector.tensor_mul(
                        dst[DH:128, :], ust2[DH:128, :], rep[DH:128, :]
                    )

                # prologue: head-pair 0 needs its full q (scores are key-major:
                # every j reads all query columns >= 128j) but only the first
                # k block; k blocks 1..3 ride along as attention fillers.
                # Chunk-major: blocks (q0,q1) cross-pair in ppa's two banks
                # and (q2,q3) in ppb's, so the PE streams every x chunk as
                # its DMA lands with adjacent MMs on opposite array halves;
                # the k block follows as a lone full-K chain.
                cq = qkp.tile([128, L], BF16, tag="qT", bufs=2)
                ck = qkp.tile([128, L], BF16, tag="kT", bufs=2)
                ppa = wkps.tile([128, 1024], F32, tag="wk", bufs=2, name="ppa")
                ppb = wkps.tile([128, 1024], F32, tag="wk", bufs=2, name="ppb")
                if OLD_PROLOGUE:
                    ppk = wkps.tile([128, 512], F32, tag="w5", bufs=4, name="ppk")
                    pjobs0 = [
                        (ppa[:, 0:512], wq_sb, 0),
                        (ppa[:, 512:1024], wq_sb, 1),
                        (ppb[:, 0:512], wq_sb, 2),
                        (ppb[:, 512:1024], wq_sb, 3),
                        (ppk[:, :], wk_sb, 0),
                    ]
                    for c in range(DC):
                        for ps, w_sb, qb in pjobs0:
                            nc.tensor.matmul(
                                ps,
                                w_sb[:, c, 0:128],
                                xT_sb[:, c, qb * 512 : (qb + 1) * 512],
                                start=(c == 0),
                                stop=(c == DC - 1),
                            )
                    nc.vector.tensor_copy(cq[:, 0:1024], ppa)
                    nc.vector.tensor_copy(cq[:, 1024:2048], ppb)
                    nc.vector.tensor_copy(ck[:, 0:512], ppk)
                else:
                    # (psum region, row-half, query block, first-of-region)
                    pjobs = [
                        (ppa[:, 0:512], 0, 0, True),
                        (ppa[:, 512:1024], 1, 1, True),
                        (ppa[:, 0:512], 1, 0, False),
                        (ppa[:, 512:1024], 0, 1, False),
                        (ppb[:, 0:512], 0, 2, True),
                        (ppb[:, 512:1024], 1, 3, True),
                        (ppb[:, 0:512], 1, 2, False),
                        (ppb[:, 512:1024], 0, 3, False),
                    ]
                    for c in range(DC):
                        for ps, half, qb, first in pjobs:
                            p0 = half * 64
                            nc.tensor.matmul(
                                ps,
                                wq_sb[p0 : p0 + 64, c, 0:128],
                                xT_sb[p0 : p0 + 64, c, qb * 512 : (qb + 1) * 512],
                                start=(c == 0 and first),
                                stop=(c == DC - 1 and not first),
                            )
                    nc.vector.tensor_copy(cq[:, 0:1024], ppa)
                    nc.vector.tensor_copy(cq[:, 1024:2048], ppb)
                    chain_full(q_spec(wk_sb, 0, 0, ck))

                if "a" in phases:
                    for hp in range(4):
                        at0 = att.tile([128, ATT_W], BF16, tag="attnT", bufs=2)
                        at1 = att.tile([128, ATT_W], BF16, tag="attnT", bufs=2)
                        atl = (at0, at1)
                        if hp < 3:
                            nq = qkp.tile([128, L], BF16, tag="qT", bufs=2)
                            nk = qkp.tile([128, L], BF16, tag="kT", bufs=2)
                        for b in range(QB):
                            # per-block micro-unit list, popped one unit per
                            # score chunk, leftovers flushed at block end.
                            # Ordering encodes the data deadlines: av(b-1)
                            # units read v chunks produced in earlier
                            # blocks; head-pair-0's own k block for b+1 and
                            # the v chunks a given av needs are always
                            # emitted in a strictly earlier list position.
                            units = []
                            if b > 1:
                                units.append(
                                    lambda hb=b - 2: norm_tail(hp, hb)
                                )
                            if b > 0:
                                units += av_units(hp, b - 1, atl, seg=2)
                            if hp == 0:
                                if b == 0:
                                    units += full_units(
                                        q_spec(wk_sb, 0, 1, ck)
                                    )
                                    units += pair_units(
                                        q_spec(wq_sb, 1, 0, nq),
                                        q_spec(wk_sb, 1, 0, nk),
                                    )
                                    units += pair_units(v_spec(0), v_spec(1))
                                    units += pair_units(v_spec(2), v_spec(3))
                                elif b == 1:
                                    units += pair_units(
                                        q_spec(wk_sb, 0, 2, ck), v_spec(4)
                                    )
                                    units += pair_units(
                                        q_spec(wq_sb, 1, 1, nq),
                                        q_spec(wk_sb, 1, 1, nk),
                                    )
                                    units += pair_units(v_spec(5), v_spec(6))
                                    units += full_units(v_spec(7))
                                elif b == 2:
                                    units += pair_units(
                                        q_spec(wk_sb, 0, 3, ck), v_spec(8)
                                    )
                                    units += pair_units(
                                        q_spec(wq_sb, 1, 2, nq),
                                        q_spec(wk_sb, 1, 2, nk),
                                    )
                                    units += pair_units(v_spec(9), v_spec(10))
                                    units += full_units(v_spec(11))
                                else:
                                    units += pair_units(
                                        q_spec(wq_sb, 1, 3, nq),
                                        q_spec(wk_sb, 1, 3, nk),
                                    )
                                    units += pair_units(
                                        v_spec(12), v_spec(13)
                                    )
                                    units += pair_units(
                                        v_spec(14), v_spec(15)
                                    )
                            elif hp < 3:
                                units += pair_units(
                                    q_spec(wq_sb, hp + 1, b, nq),
                                    q_spec(wk_sb, hp + 1, b, nk),
                                )
                            if hp == 3 and "o" in phases and b == 3:
                                units += [op_unit(qt) for qt in range(4)]
                            leftovers = sc_group(hp, b, atl, cq, ck, units)
                            for f in leftovers:
                                f()
                        for f in av_units(hp, 3, atl, seg=4):
                            f()
                        norm_tail(hp, 2)
                        if hp == 3 and "o" in phases:
                            for qt in range(4, 8):
                                op_unit(qt)()
                        norm_tail(hp, 3)
                        if hp < 3:
                            cq, ck = nq, nk
                    if "o" in phases:
                        for qt in range(8, 16):
                            op_unit(qt)()
                else:
                    # keep projections live when attention is ablated
                    nc.sync.dma_start(
                        out=out_r[:, 0, 0:512], in_=cq[:, 0:512]
                    )

        if reps > 0:
            with tc.For_i(0, reps):
                body()
        body()


_CACHE = {}


def _get_runner(reps=0):
    if reps not in _CACHE:
        _CACHE[reps] = build_kernel(reps)
    return _CACHE[reps]


def make_in_maps(x, Wq, Wk, Wv, Wo):
    in_maps = []
    for core in range(N_CORES):
        b, hg = divmod(core, 2)
        sl = slice(hg * DHG, (hg + 1) * DHG)
        in_maps.append(
            {
                "xT": np.ascontiguousarray(np.asarray(x)[b].T.astype(bfloat16)),
                "wqT": np.ascontiguousarray(np.asarray(Wq)[sl, :].T.astype(bfloat16)),
                "wkT": np.ascontiguousarray(np.asarray(Wk)[sl, :].T.astype(bfloat16)),
                "wvT": np.ascontiguousarray(np.asarray(Wv)[sl, :].T.astype(bfloat16)),
                "woT": np.ascontiguousarray(np.asarray(Wo)[:, sl].T.astype(bfloat16)),
            }
        )
    return in_maps


def kernel(x, Wq, Wk, Wv, Wo):
    x = np.asarray(x)
    nc = _get_runner(0)
    in_maps = make_in_maps(x, Wq, Wk, Wv, Wo)
    res = run_bass_kernel_spmd(nc, in_maps, core_ids=list(range(N_CORES)))
    B = x.shape[0]
    out = np.empty((B, L, D), dtype=np.float32)
    for b in range(B):
        out[b] = res.results[2 * b]["out"].astype(np.float32) + res.results[
            2 * b + 1
        ]["out"].astype(np.float32)
    return out
